# revision 10
# baseline (speedup 1.0000x reference)
"""Trainium2 Bass kernel for nn_MultiHeadAttention_38027640439053.

Reference computation (per batch b of 8, one NeuronCore each):
    data = X.reshape(n, 16, 64)
    q/k/v = data @ W{q,k,v}.T          (per-head shared 64x64 weights)
    scores = (q @ k.T per head) / 32
    attn = softmax(scores, axis=k)
    Y = (attn @ v).reshape(n, 1024) @ Wo.T + bo

V2 strategy (batch-parallel over 8 cores, zero collectives, bf16):
  - X is converted to bf16 on the host; each pair-of-heads column slab
    is loaded TRANSPOSED directly from DRAM via the XBAR DMA transpose
    (dma_start_transpose) -> no PE transposes at all.
  - Q and K projections are FUSED: scores = X A X^T with
    A = Wq^T Wk (64x64, shared by all heads).  Only one projected
    operand (GT = A2^T @ XT) is needed; the score matmul's stationary
    side is raw XT.  Two heads run concurrently in the PE via
    row-tiling (partition bases 0 / 64).
  - exp on ScalarE from PSUM with the 1/32 scale folded in, output
    straight to bf16 PT tiles.
  - P@V transposed with a ones-augmented V (row 64 of pvps = softmax
    denominator).  Reciprocal runs on DVE directly from PSUM into a
    persistent [65, N] tile (rows 0/64); a [65->128] selector matmul
    broadcasts 1/D across partitions; one DVE multiply per pair.
  - Wo^T is preloaded in bf16 at program start; output projection
    accumulates 8 pair-chunks + rank-1 bias into PSUM per n-tile.
  - Software pipelining identical in spirit to v1: previous pair's PV
    and next pair's loads/projections are interleaved into the current
    pair's ACT-bound score/exp loop.
"""

import numpy as np
import ml_dtypes

import concourse.bacc as bacc
import concourse.mybir as mybir
import concourse.tile as tile
from concourse.bass_utils import run_bass_kernel_spmd

F32 = mybir.dt.float32
BF16 = mybir.dt.bfloat16
I16 = mybir.dt.int16

EXP = mybir.ActivationFunctionType.Exp

# (ktile, head) score tiles whose exp runs on the DVE via the Schraudolph
# bit trick (j = int16(s*A + B); bits reinterpreted as bf16 ~= 2^(s*log2e)).
# Balances the ScalarE exp load against idle DVE capacity; each DVE tile
# adds ~3% sawtooth error to its attention weights (end-to-end rel err
# measured 8.4e-3 vs the 2e-2 gate).
DVE_EXP = frozenset({(1, 0), (3, 1), (5, 0), (7, 1)})
SCHR_A = 128.0 * float(np.log2(np.e))   # per unit *scaled* score
SCHR_B = 128.0 * (127.0 - 0.0434)


def emit_body(tc, nc, aps, N, EMB, NH, rep):
    NPAIR = NH // 2
    NT = N // 128        # n tiles (rows of X / q tiles)
    KT = N // 128        # k tiles
    assert EMB == NPAIR * 128
    scale = 1.0 / float(np.sqrt(EMB))
    qch = [(s, min(512, N - s)) for s in range(0, N, 512)]
    ech = [(s, min(512, EMB - s)) for s in range(0, EMB, 512)]

    X_d, A2_d, Wv2_d, WoT_d, bo_d, sel_d, ones_d, Y_d = aps

    with (
        tc.tile_pool(name=f"consts{rep}", bufs=1) as consts,
        tc.tile_pool(name=f"xtp{rep}", bufs=3) as xtp,
        tc.tile_pool(name=f"gtp{rep}", bufs=2) as gtp,
        tc.tile_pool(name=f"vp{rep}", bufs=3) as vp,
        tc.tile_pool(name=f"ptp{rep}", bufs=3) as ptp,
        tc.tile_pool(name=f"ytp{rep}", bufs=NPAIR) as ytp,
        tc.tile_pool(name=f"rdp{rep}", bufs=1) as rdp,
        tc.tile_pool(name=f"osbp{rep}", bufs=2) as osbp,
        tc.tile_pool(name=f"stps{rep}", bufs=2, space="PSUM") as stps,
        tc.tile_pool(name=f"mps{rep}", bufs=2, space="PSUM") as mps,
    ):
        # ---- constants needed immediately (tiny, ahead of xt0 on SP) ----
        a2 = consts.tile([128, 128], BF16, name="a2", tag="a2")
        nc.sync.dma_start(out=a2[:], in_=A2_d[:])
        wv2 = consts.tile([128, 128], BF16, name="wv2", tag="wv2")
        nc.sync.dma_start(out=wv2[:], in_=Wv2_d[:])

        # persistent denominator tile: rows 0 / 64 hold 1/D of the current
        # pair's two heads; all other rows stay zero forever.
        ds = rdp.tile([65, N], BF16, name="ds", tag="ds")
        nc.gpsimd.memset(ds[:], 0.0)

        # ---- late-need constants (declared here, DMAs emitted after the
        # pair-0 prologue so they queue behind xt0 on SP) ----
        bo_t = consts.tile([1, EMB], BF16, name="bo_t", tag="bo_t")
        ones_t = consts.tile([1, 128], BF16, name="ones_t", tag="ones_t")
        sel_t = consts.tile([65, 128], BF16, name="sel_t", tag="sel_t")
        wot = consts.tile([128, NPAIR * EMB], BF16, name="wot", tag="wot")

        def load_late_consts():
            nc.sync.dma_start(out=sel_t[:], in_=sel_d[:])
            nc.sync.dma_start(out=bo_t[:], in_=bo_d[:])
            nc.sync.dma_start(out=ones_t[:], in_=ones_d[:])
            nc.sync.dma_start(
                out=wot[:].rearrange("p (c e) -> p c e", e=EMB),
                in_=WoT_d[:].rearrange("(c p) e -> p c e", p=128))

        # ---- per-pair helpers ----
        xts = {}

        def load_xt(p):
            xt = xtp.tile([128, N], BF16, name=f"xt{p}", tag="xt")
            nc.sync.dma_start_transpose(
                out=xt[:], in_=X_d[:, p * 128:(p + 1) * 128])
            xts[p] = xt

        def proj_gt(p):
            xt = xts[p]
            gps = mps.tile([128, N], F32, name=f"gps{p}", tag="m")
            for (s, w) in qch:
                nc.tensor.matmul(gps[:, s:s + w], a2[:], xt[:, s:s + w])
            gt = gtp.tile([128, N], BF16, name=f"gt{p}", tag="gt")
            nc.vector.tensor_copy(gt[:], gps[:])
            return gt

        def proj_v(p):
            xt = xts[p]
            vps = mps.tile([128, N], F32, name=f"vps{p}", tag="m")
            for i in range(NT):
                nc.tensor.matmul(vps[:, i * 128:(i + 1) * 128],
                                 xt[:, i * 128:(i + 1) * 128], wv2[:])
            vslab = vp.tile([128, KT * 130], BF16, name=f"vslab{p}", tag="v")
            v4 = vslab[:].rearrange("p (j k c) -> p j k c", k=2, c=65)
            vs4 = vps[:].rearrange("p (j k c) -> p j k c", k=2, c=64)
            nc.vector.tensor_copy(v4[:, :, :, 0:64], vs4[:])
            nc.gpsimd.memset(v4[:, :, :, 64:65], 1.0)
            return vslab

        def st_exp(p, ktile, gt, pt):
            """Transposed scores + exp for one k-tile, both heads."""
            xt = xts[p]
            for head in (0, 1):
                r0 = head * 64
                st = stps.tile([128, N], F32, name=f"st{p}_{ktile}_{head}",
                               tag="st")
                for (s, w) in qch:
                    nc.tensor.matmul(
                        st[:, s:s + w],
                        xt[r0:r0 + 64, ktile * 128:(ktile + 1) * 128],
                        gt[r0:r0 + 64, s:s + w],
                    )
                dst = pt[:, (ktile * 2 + head) * N:(ktile * 2 + head + 1) * N]
                if (ktile, head) in DVE_EXP:
                    with nc.allow_low_precision(reason="schraudolph exp"):
                        nc.vector.tensor_scalar(
                            dst.bitcast(I16), st[:],
                            SCHR_A * scale, SCHR_B,
                            mybir.AluOpType.mult, mybir.AluOpType.add)
                else:
                    nc.scalar.activation(dst, st[:], EXP, scale=scale)

        pv_state = {}

        def pv_half(p, head, half, vslab, pt):
            """8 accumulating matmuls: k-tiles [half*KT/2, (half+1)*KT/2)."""
            if half == 0:
                pv_state[(p, head)] = mps.tile(
                    [65, N], F32, name=f"pvps{p}_{head}", tag="m")
            pvps = pv_state[(p, head)]
            k0, k1 = half * (KT // 2), (half + 1) * (KT // 2)
            for ktile in range(k0, k1):
                lhs = vslab[:, ktile * 130 + head * 65:
                            ktile * 130 + head * 65 + 65]
                base = (ktile * 2 + head) * N
                for (s, w) in qch:
                    nc.tensor.matmul(
                        pvps[:, s:s + w], lhs,
                        pt[:, base + s:base + s + w],
                        start=(ktile == 0), stop=(ktile == KT - 1),
                    )

        def finish_head(p, head, yt):
            pvps = pv_state.pop((p, head))
            nc.vector.tensor_copy(yt[head * 64:head * 64 + 64, :],
                                  pvps[0:64, :])
            with nc.allow_low_precision(reason="bf16 softmax denom"):
                nc.vector.reciprocal(ds[head * 64:head * 64 + 1, :],
                                     pvps[64:65, :])

        def bcast_mul(p, yt):
            bps = mps.tile([128, N], F32, name=f"bps{p}", tag="m")
            for (s, w) in qch:
                nc.tensor.matmul(bps[:, s:s + w], sel_t[:], ds[:, s:s + w])
            nc.vector.tensor_mul(yt[:], yt[:], bps[:])

        # ---------------- pipelined pair loop ----------------
        yts = []
        pts = {}
        vslabs = {}

        load_xt(0)
        cur_gt = proj_gt(0)
        vslabs[0] = proj_v(0)
        load_late_consts()
        nxt = {}
        for p in range(NPAIR):
            pt = ptp.tile([128, KT * 2 * N], BF16, name=f"pt{p}", tag="pt")
            pts[p] = pt
            yts.append(ytp.tile([128, N], BF16, name=f"yt{p}", tag="yt"))

            sched = {k: [] for k in range(KT)}
            if p > 0:
                po, vo, pp = p - 1, vslabs[p - 1], pts[p - 1]
                yo = yts[p - 1]
                tasks = [
                    lambda: pv_half(po, 0, 0, vo, pp),
                    lambda: (pv_half(po, 0, 1, vo, pp),
                             finish_head(po, 0, yo)),
                    lambda: pv_half(po, 1, 0, vo, pp),
                    lambda: (pv_half(po, 1, 1, vo, pp),
                             finish_head(po, 1, yo), bcast_mul(po, yo)),
                ]
                for j, pos in enumerate((0, KT // 4, KT // 2,
                                         (3 * KT) // 4)):
                    sched[min(KT - 1, pos)].append(tasks[j])
            if p + 1 < NPAIR:
                pn = p + 1
                tasks = [
                    lambda: load_xt(pn),
                    lambda: nxt.__setitem__("gt", proj_gt(pn)),
                    lambda: vslabs.__setitem__(pn, proj_v(pn)),
                ]
                for j, pos in enumerate((0, KT - 5, KT - 3)):
                    sched[max(0, pos)].append(tasks[j])
            else:
                # eager PV for the last pair: first halves (k-tiles 0-3)
                # only — their pt slices are already emitted by then.
                sched[KT // 2].append(
                    lambda: pv_half(p, 0, 0, vslabs[p], pts[p]))
                sched[KT - 1].append(
                    lambda: pv_half(p, 1, 0, vslabs[p], pts[p]))
            for ktile in range(KT):
                for t in sched[ktile]:
                    t()
                st_exp(p, ktile, cur_gt, pt)
            if p - 1 >= 0:
                del vslabs[p - 1], pts[p - 1]
            if p + 1 < NPAIR:
                cur_gt = nxt["gt"]

        # ---------------- tail: last pair's PV + outproj ----
        last = NPAIR - 1
        pv_half(last, 0, 1, vslabs[last], pts[last])
        finish_head(last, 0, yts[last])
        pv_half(last, 1, 1, vslabs[last], pts[last])
        finish_head(last, 1, yts[last])
        bcast_mul(last, yts[last])

        for i in range(NT):
            ops = mps.tile([128, EMB], F32, name=f"ops{i}", tag="m")
            for p in range(NPAIR):
                for (s, w) in ech:
                    nc.tensor.matmul(
                        ops[:, s:s + w],
                        yts[p][:, i * 128:(i + 1) * 128],
                        wot[:, p * EMB + s:p * EMB + s + w],
                        start=(p == 0), stop=False,
                    )
            for (s, w) in ech:
                nc.tensor.matmul(ops[:, s:s + w], ones_t[:], bo_t[:, s:s + w],
                                 start=False, stop=True)
            osb = osbp.tile([128, EMB], F32, name=f"osb{i}", tag="osb")
            nc.vector.tensor_copy(osb[:], ops[:])
            nc.sync.dma_start(out=Y_d[i * 128:(i + 1) * 128, :], in_=osb[:])


def build_program(N=1024, EMB=1024, NH=16, n_cores=8, repeat=1,
                  trace_sim=False):
    nc = bacc.Bacc("TRN2", target_bir_lowering=False, debug=False,
                   num_devices=n_cores)
    aps = (
        nc.dram_tensor("X", [N, EMB], BF16, kind="ExternalInput").ap(),
        nc.dram_tensor("A2", [128, 128], BF16, kind="ExternalInput").ap(),
        nc.dram_tensor("Wv2", [128, 128], BF16, kind="ExternalInput").ap(),
        nc.dram_tensor("WoT", [EMB, EMB], BF16, kind="ExternalInput").ap(),
        nc.dram_tensor("bo", [1, EMB], BF16, kind="ExternalInput").ap(),
        nc.dram_tensor("sel", [65, 128], BF16, kind="ExternalInput").ap(),
        nc.dram_tensor("ones", [1, 128], BF16, kind="ExternalInput").ap(),
        nc.dram_tensor("Y", [N, EMB], F32, kind="ExternalOutput").ap(),
    )
    with tile.TileContext(nc, trace_sim=trace_sim) as tc:
        for rep in range(repeat):
            emit_body(tc, nc, aps, N, EMB, NH, rep)
    nc.compile()
    return nc


def host_consts(Wq, Wk, Wv, Wo, bo, NH=16):
    EMB = NH * 64
    bf = ml_dtypes.bfloat16

    A = np.asarray(Wq, np.float32).T @ np.asarray(Wk, np.float32)

    def blk2(B):
        out = np.zeros((128, 128), np.float32)
        out[0:64, 0:64] = B
        out[64:128, 64:128] = B
        return out

    # selector: row 0 -> output partitions 0..63, row 64 -> 64..127
    sel = np.zeros((65, 128), np.float32)
    sel[0, 0:64] = 1.0
    sel[64, 64:128] = 1.0
    return {
        "A2": blk2(A).astype(bf),
        "Wv2": blk2(np.asarray(Wv, np.float32).T).astype(bf),
        "WoT": np.ascontiguousarray(
            np.asarray(Wo, np.float32).T).astype(bf),
        "bo": np.asarray(bo, np.float32).reshape(1, EMB).astype(bf),
        "sel": sel.astype(bf),
        "ones": np.ones((1, 128), np.float32).astype(bf),
    }


def stage_x(X_core):
    """Convert one core's [N, EMB] fp32 activation slab to bf16."""
    return np.ascontiguousarray(
        np.asarray(X_core, np.float32).astype(ml_dtypes.bfloat16))


_NC_CACHE = {}


def kernel(X, Wq, Wk, Wv, Wo, bo):
    X = np.asarray(X, np.float32)
    B, N, EMB = X.shape
    NH = EMB // 64
    key = (N, EMB, NH, B)
    if key not in _NC_CACHE:
        _NC_CACHE[key] = build_program(N=N, EMB=EMB, NH=NH, n_cores=B)
    nc = _NC_CACHE[key]
    consts = host_consts(Wq, Wk, Wv, Wo, bo, NH=NH)
    in_maps = [dict(consts, X=stage_x(X[c])) for c in range(B)]
    res = run_bass_kernel_spmd(nc, in_maps, list(range(B)))
    return np.stack([res.results[c]["Y"] for c in range(B)], axis=0)


if __name__ == "__main__":
    rng = np.random.default_rng(0)
    B, N, EMB, NH = 8, 1024, 1024, 16
    X = rng.standard_normal((B, N, EMB), dtype=np.float32)
    Wq = (rng.standard_normal((64, 64), dtype=np.float32) / 8)
    Wk = (rng.standard_normal((64, 64), dtype=np.float32) / 8)
    Wv = (rng.standard_normal((64, 64), dtype=np.float32) / 8)
    Wo = (rng.standard_normal((EMB, EMB), dtype=np.float32) / 32)
    bo = np.zeros(EMB, np.float32)
    Y = kernel(X=X, Wq=Wq, Wk=Wk, Wv=Wv, Wo=Wo, bo=bo)
    print("OK", Y.shape, Y.dtype)
